# revision 3
# baseline (speedup 1.0000x reference)
import sys

sys.path.insert(0, "/opt/trn_rl_repo")
import numpy as np
import ml_dtypes

import concourse.bass as bass
import concourse.tile as tile
import concourse.bacc as bacc
from concourse import mybir
from concourse.bass_utils import run_bass_kernel_spmd

BF16 = mybir.dt.bfloat16
F32 = mybir.dt.float32

N_CORES = 8
EMBED = 768
BLOCKS = 8
BS = 96
LATENT = 4 * EMBED  # 3072
LAMBD = 0.01
EPS = 1e-5
H = 128
W = 128
WF = 65  # rfft width

# Device problem sizes (per core)
S1 = (H * W) // N_CORES          # 2048 spatial pixels per core (MLP ss_cnn)
S2 = 1024                        # padded spectral pixels per core (8320/8=1040 -> 2*512=1024 after repack)
SPEC_TOT = H * WF                # 8320
SPEC_PAD = S2 * N_CORES          # 8192 < 8320!  -> use 1040 -> pad to 1024? NO. recompute below.

# 8320 spectral pixels over 8 cores = 1040 each; pad each core's slice to 1152? must be mult of 512 for psum blocks
S2 = 1536  # 3 blocks of 512; 8*1536=12288 >= 8320 with padding
SBLK = 512


def _erf(x):
    # Abramowitz & Stegun 7.1.26, |err| <= 1.5e-7
    a1, a2, a3, a4, a5, p = (
        0.254829592, -0.284496736, 1.421413741, -1.453152027, 1.061405429, 0.3275911,
    )
    s = np.sign(x)
    ax = np.abs(x)
    t = 1.0 / (1.0 + p * ax)
    y = 1.0 - (((((a5 * t + a4) * t) + a3) * t + a2) * t + a1) * t * np.exp(-ax * ax)
    return s * y


def _gelu(x):
    return 0.5 * x * (1.0 + _erf(x / np.sqrt(2.0)))


def _layernorm(x, w, b):
    m = x.mean(-1, keepdims=True)
    v = x.var(-1, keepdims=True)
    return (x - m) / np.sqrt(v + EPS) * w + b


def _softshrink(x, l):
    return np.where(x > l, x - l, np.where(x < -l, x + l, 0.0)).astype(np.float32)


def _blockmm(x, w):
    return np.einsum("nyxbi,bio->nyxbo", x, w, optimize=True)


def _conv_pipeline(nc, tc, ctx, tag, A, W1p, B1, W2p, B2, H1, OUT, c_in, hid, nout, s_tot):
    """out = W2 @ relu(W1 @ A + b1) + b2, channels on partitions, pixels on free dim.

    A: DRAM [c_in, s_tot] bf16. W1p: [hid//128, c_in, 128] bf16. B1: [hid, 1] f32.
    W2p: [nout//128, hid//128, 128, 128] bf16. B2: [nout, 1] f32.
    H1: DRAM scratch [hid, s_tot] bf16. OUT: DRAM [nout, s_tot] bf16.
    """
    from contextlib import ExitStack

    cc = c_in // 128
    hc = hid // 128
    oc = nout // 128
    nb = s_tot // SBLK

    ctx = ExitStack()  # pools freed at end of this pipeline so stages don't stack in SBUF
    ap = ctx.enter_context(tc.tile_pool(name=f"{tag}_a", bufs=1))
    wp = ctx.enter_context(tc.tile_pool(name=f"{tag}_w", bufs=2))
    hp = ctx.enter_context(tc.tile_pool(name=f"{tag}_h", bufs=3))
    bp = ctx.enter_context(tc.tile_pool(name=f"{tag}_b", bufs=2))
    pp = ctx.enter_context(tc.tile_pool(name=f"{tag}_p", bufs=4, space="PSUM"))

    # A resident: [128, cc, s_tot]
    at = ap.tile([128, cc, s_tot], BF16)
    nc.sync.dma_start(at[:], A.rearrange("(c p) s -> p c s", p=128))

    # conv1: stream W1 strips; h1 -> DRAM
    def conv1_body(i):
        w1t = wp.tile([128, cc, 128], BF16, tag="w1")
        nc.sync.dma_start(w1t[:], W1p[bass.ds(i, 1), :, :].rearrange("one (c p) m -> p (one c) m", p=128))
        b1t = bp.tile([128, 1], F32, tag="b1")
        nc.sync.dma_start(b1t[:], B1[bass.ds(i * 128, 128), :])
        for sb in range(nb):
            ps = pp.tile([128, SBLK], F32, tag="ps1")
            for c in range(cc):
                nc.tensor.matmul(
                    ps[:], w1t[:, c, :], at[:, c, bass.ts(sb, SBLK)],
                    start=(c == 0), stop=(c == cc - 1),
                )
            h1t = hp.tile([128, SBLK], BF16, tag="h1")
            nc.scalar.activation(h1t[:], ps[:], mybir.ActivationFunctionType.Relu, bias=b1t[:, 0:1])
            nc.sync.dma_start(H1[bass.ds(i * 128, 128), bass.ts(sb, SBLK)], h1t[:])

    tc.For_i_unrolled(0, hc, 1, conv1_body, max_unroll=4)

    # conv2: per s-block keep h1 block resident, stream W2 strips
    for sb in range(nb):
        h1b = ap.tile([128, hc, SBLK], BF16, tag="h1b")
        nc.sync.dma_start(
            h1b[:],
            H1[:, bass.ts(sb, SBLK)].rearrange("(k p) s -> p k s", p=128),
        )
        def conv2_body(o, sb=sb, h1b=h1b):
            w2t = wp.tile([128, hc, 128], BF16, tag="w2")
            nc.sync.dma_start(
                w2t[:], W2p[bass.ds(o, 1), :, :, :].rearrange("one k p m -> p (one k) m")
            )
            b2t = bp.tile([128, 1], F32, tag="b2")
            nc.sync.dma_start(b2t[:], B2[bass.ds(o * 128, 128), :])
            ps2 = pp.tile([128, SBLK], F32, tag="ps2")
            for k in range(hc):
                nc.tensor.matmul(
                    ps2[:], w2t[:, k, :], h1b[:, k, :],
                    start=(k == 0), stop=(k == hc - 1),
                )
            ot = hp.tile([128, SBLK], BF16, tag="ot")
            nc.vector.tensor_scalar_add(ot[:], ps2[:], b2t[:, 0:1])
            nc.sync.dma_start(OUT[bass.ds(o * 128, 128), bass.ts(sb, SBLK)], ot[:])

        tc.For_i_unrolled(0, oc, 1, conv2_body, max_unroll=2)
    ctx.close()


_PROGRAM = None
LAST_RESULT = None


def _build_program():
    global _PROGRAM
    if _PROGRAM is not None:
        return _PROGRAM
    nc = bacc.Bacc("TRN2", target_bir_lowering=False, debug=False, num_devices=N_CORES)

    a1 = nc.dram_tensor("a1", [EMBED, S1], BF16, kind="ExternalInput")
    a2 = nc.dram_tensor("a2", [EMBED, S2], BF16, kind="ExternalInput")
    a3 = nc.dram_tensor("a3", [EMBED, S2], BF16, kind="ExternalInput")
    w1a = nc.dram_tensor("w1a", [4 * LATENT // 128, EMBED, 128], BF16, kind="ExternalInput")
    b1a = nc.dram_tensor("b1a", [4 * LATENT, 1], F32, kind="ExternalInput")
    w2a = nc.dram_tensor("w2a", [2 * LATENT // 128, 4 * LATENT // 128, 128, 128], BF16, kind="ExternalInput")
    b2a = nc.dram_tensor("b2a", [2 * LATENT, 1], F32, kind="ExternalInput")
    w1f = nc.dram_tensor("w1f", [4 * EMBED // 128, EMBED, 128], BF16, kind="ExternalInput")
    b1f = nc.dram_tensor("b1f", [4 * EMBED, 1], F32, kind="ExternalInput")
    w2f = nc.dram_tensor("w2f", [2 * EMBED // 128, 4 * EMBED // 128, 128, 128], BF16, kind="ExternalInput")
    b2f = nc.dram_tensor("b2f", [2 * EMBED, 1], F32, kind="ExternalInput")

    o1 = nc.dram_tensor("o1", [2 * LATENT, S1], BF16, kind="ExternalOutput")
    o2 = nc.dram_tensor("o2", [2 * EMBED, S2], BF16, kind="ExternalOutput")
    o3 = nc.dram_tensor("o3", [2 * EMBED, S2], BF16, kind="ExternalOutput")

    h1a = nc.dram_tensor("h1a", [4 * LATENT, S1], BF16, kind="Internal")
    h1f2 = nc.dram_tensor("h1f2", [4 * EMBED, S2], BF16, kind="Internal")
    h1f3 = nc.dram_tensor("h1f3", [4 * EMBED, S2], BF16, kind="Internal")

    from contextlib import ExitStack

    with tile.TileContext(nc) as tc, ExitStack() as ctx:
        _conv_pipeline(nc, tc, ctx, "m", a1, w1a, b1a, w2a, b2a, h1a, o1,
                       EMBED, 4 * LATENT, 2 * LATENT, S1)
        _conv_pipeline(nc, tc, ctx, "fr", a2, w1f, b1f, w2f, b2f, h1f2, o2,
                       EMBED, 4 * EMBED, 2 * EMBED, S2)
        _conv_pipeline(nc, tc, ctx, "fi", a3, w1f, b1f, w2f, b2f, h1f3, o3,
                       EMBED, 4 * EMBED, 2 * EMBED, S2)
    nc.compile()
    _PROGRAM = nc
    return nc


def _bf16(x):
    return np.ascontiguousarray(x).astype(ml_dtypes.bfloat16)


def kernel(x, mod_embed, norm1_w, norm1_b, norm2_w, norm2_b, w1, b1, w2, b2,
           f_c1_w, f_c1_b, f_c2_w, f_c2_b, fc1_w, fc1_b, fc2_w, fc2_b,
           m_c1_w, m_c1_b, m_c2_w, m_c2_b):
    x = np.asarray(x, np.float32)
    mod_embed = np.asarray(mod_embed, np.float32)
    B = x.shape[0]
    assert B == 1 and x.shape == (1, H, W, EMBED)

    # ---- host: LN1 + forward FFTs (cheap) ----
    residual = x
    xn = _layernorm(x, np.asarray(norm1_w, np.float32), np.asarray(norm1_b, np.float32))
    xf = np.fft.rfft2(xn[0].astype(np.float64), axes=(0, 1), norm="ortho")  # [H, WF, C]
    mf = np.fft.rfft2(np.asarray(mod_embed[0], np.float64), axes=(0, 1), norm="ortho")
    mr = np.ascontiguousarray(mf.real.astype(np.float32))  # [H, WF, C]
    mi = np.ascontiguousarray(mf.imag.astype(np.float32))

    # ---- device: the three conv pipelines ----
    nc = _build_program()

    # spatial pixels -> 8 shards of 2048 (by H rows: 16 rows each)
    modp = mod_embed[0].reshape(H * W, EMBED)  # [16384, 768]
    # spectral pixels flattened [H*WF, C] = 8320 rows, pad to 8*S2
    mr_f = mr.reshape(SPEC_TOT, EMBED)
    mi_f = mi.reshape(SPEC_TOT, EMBED)
    pad = N_CORES * S2 - SPEC_TOT
    mr_p = np.concatenate([mr_f, np.zeros((pad, EMBED), np.float32)], 0)
    mi_p = np.concatenate([mi_f, np.zeros((pad, EMBED), np.float32)], 0)

    w1a_h = _bf16(np.asarray(m_c1_w, np.float32).T.reshape(EMBED, 4 * LATENT // 128, 128).transpose(1, 0, 2))
    w2a_h = _bf16(
        np.asarray(m_c2_w, np.float32).T  # [4L, 2L]
        .reshape(4 * LATENT // 128, 128, 2 * LATENT // 128, 128)
        .transpose(2, 0, 1, 3)
    )
    w1f_h = _bf16(np.asarray(f_c1_w, np.float32).T.reshape(EMBED, 4 * EMBED // 128, 128).transpose(1, 0, 2))
    w2f_h = _bf16(
        np.asarray(f_c2_w, np.float32).T
        .reshape(4 * EMBED // 128, 128, 2 * EMBED // 128, 128)
        .transpose(2, 0, 1, 3)
    )
    shared = {
        "w1a": w1a_h, "b1a": np.asarray(m_c1_b, np.float32).reshape(-1, 1),
        "w2a": w2a_h, "b2a": np.asarray(m_c2_b, np.float32).reshape(-1, 1),
        "w1f": w1f_h, "b1f": np.asarray(f_c1_b, np.float32).reshape(-1, 1),
        "w2f": w2f_h, "b2f": np.asarray(f_c2_b, np.float32).reshape(-1, 1),
    }
    in_maps = []
    for k in range(N_CORES):
        m = dict(shared)
        m["a1"] = _bf16(modp[k * S1:(k + 1) * S1].T)
        m["a2"] = _bf16(mr_p[k * S2:(k + 1) * S2].T)
        m["a3"] = _bf16(mi_p[k * S2:(k + 1) * S2].T)
        in_maps.append(m)

    res = run_bass_kernel_spmd(nc, in_maps, core_ids=list(range(N_CORES)))
    global LAST_RESULT
    LAST_RESULT = res

    # reassemble
    ss_mlp = np.maximum(np.concatenate(
        [res.results[k]["o1"].astype(np.float32).T for k in range(N_CORES)], 0
    ), 0.0)  # [16384, 6144], second relu of _ss_cnn
    fo_re = np.maximum(np.concatenate(
        [res.results[k]["o2"].astype(np.float32).T for k in range(N_CORES)], 0
    )[:SPEC_TOT], 0.0)  # [8320, 1536]
    fo_im = np.maximum(np.concatenate(
        [res.results[k]["o3"].astype(np.float32).T for k in range(N_CORES)], 0
    )[:SPEC_TOT], 0.0)

    # ---- host: rest of the filter ----
    xr = xf.real.astype(np.float32).reshape(1, H, WF, BLOCKS, BS)
    xi = xf.imag.astype(np.float32).reshape(1, H, WF, BLOCKS, BS)
    w1_ = np.asarray(w1, np.float32)
    b1_ = np.asarray(b1, np.float32)
    w2_ = np.asarray(w2, np.float32)
    b2_ = np.asarray(b2, np.float32)
    o1_re = _blockmm(xr, w1_[0]) - _blockmm(xi, w1_[1]) + b1_[0]
    o1_im = _blockmm(xi, w1_[0]) + _blockmm(xr, w1_[1]) + b1_[1]

    sc_re = 1.0 + fo_re[:, :EMBED].reshape(1, H, WF, BLOCKS, BS)
    sh_re = fo_re[:, EMBED:].reshape(1, H, WF, BLOCKS, BS)
    sc_im = 1.0 + fo_im[:, :EMBED].reshape(1, H, WF, BLOCKS, BS)
    sh_im = fo_im[:, EMBED:].reshape(1, H, WF, BLOCKS, BS)

    n_re = o1_re * sc_re - o1_im * sc_im + sh_re
    n_im = o1_im * sc_re + o1_re * sc_im + sh_im
    o1_re = np.maximum(n_re, 0.0)
    o1_im = np.maximum(n_im, 0.0)

    o2_re = _blockmm(o1_re, w2_[0]) - _blockmm(o1_im, w2_[1]) + b2_[0]
    o2_im = _blockmm(o1_im, w2_[0]) + _blockmm(o1_re, w2_[1]) + b2_[1]
    o2_re = _softshrink(o2_re, LAMBD)
    o2_im = _softshrink(o2_im, LAMBD)

    spec = (o2_re + 1j * o2_im).reshape(H, WF, EMBED)
    filt = np.fft.irfft2(spec, s=(H, W), axes=(0, 1), norm="ortho").astype(np.float32)
    h_mid = filt[None] + xn + residual  # filter bias (xn) + double_skip residual

    # ---- host: second half (device did scale/shift) ----
    h2 = _layernorm(h_mid, np.asarray(norm2_w, np.float32), np.asarray(norm2_b, np.float32))
    scale = 1.0 + ss_mlp[:, :LATENT].reshape(1, H, W, LATENT)
    shift = ss_mlp[:, LATENT:].reshape(1, H, W, LATENT)
    hh = h2.reshape(H * W, EMBED) @ np.asarray(fc1_w, np.float32).T + np.asarray(fc1_b, np.float32)
    hh = hh.reshape(1, H, W, LATENT) * scale + shift
    hh = _gelu(hh)
    out = hh.reshape(H * W, LATENT) @ np.asarray(fc2_w, np.float32).T + np.asarray(fc2_b, np.float32)
    return (out.reshape(1, H, W, EMBED) + h_mid).astype(np.float32)



# revision 4
# speedup vs baseline: 3.4336x; 3.4336x over previous
import sys

sys.path.insert(0, "/opt/trn_rl_repo")
import numpy as np
import ml_dtypes

import concourse.bass as bass
import concourse.tile as tile
import concourse.bacc as bacc
from concourse import mybir
from concourse.bass_utils import run_bass_kernel_spmd

BF16 = mybir.dt.bfloat16
F32 = mybir.dt.float32
FP8 = mybir.dt.float8e4
DR = mybir.MatmulPerfMode.DoubleRow
RELU = mybir.ActivationFunctionType.Relu

N_CORES = 8
EMBED = 768
KC = 6            # 768 / 128 contraction chunks
BLOCKS = 8
BS = 96
LATENT = 4 * EMBED            # 3072
HID_M = 4 * LATENT            # 12288
OUT_M = 2 * LATENT            # 6144
HID_F = 4 * EMBED             # 3072
OUT_F = 2 * EMBED             # 1536
LAMBD = 0.01
EPS = 1e-5
H = 128
W = 128
WF = 65
SPEC_TOT = H * WF             # 8320
S1 = (H * W) // N_CORES       # 2048 spatial px per core
S2 = SPEC_TOT // N_CORES      # 1040 spectral px per core
PXF = 2 * S2                  # 2080 (re ++ im)
NBF = 5
BLKF = PXF // NBF             # 416 (psum-bank safe)


def _erf(x):
    a1, a2, a3, a4, a5, p = (
        0.254829592, -0.284496736, 1.421413741, -1.453152027, 1.061405429, 0.3275911,
    )
    s = np.sign(x)
    ax = np.abs(x)
    t = 1.0 / (1.0 + p * ax)
    y = 1.0 - (((((a5 * t + a4) * t) + a3) * t + a2) * t + a1) * t * np.exp(-ax * ax)
    return s * y


def _gelu(x):
    return 0.5 * x * (1.0 + _erf(x / np.sqrt(2.0)))


def _layernorm(x, w, b):
    m = x.mean(-1, keepdims=True)
    v = x.var(-1, keepdims=True)
    return (x - m) / np.sqrt(v + EPS) * w + b


def _softshrink(x, l):
    return np.where(x > l, x - l, np.where(x < -l, x + l, 0.0)).astype(np.float32)


def _blockmm(x, w):
    return np.einsum("nyxbi,bio->nyxbo", x, w, optimize=True)


_PROGRAM = None
LAST_RESULT = None


def _build_program():
    global _PROGRAM
    if _PROGRAM is not None:
        return _PROGRAM
    from contextlib import ExitStack

    nc = bacc.Bacc("TRN2", target_bir_lowering=False, debug=False, num_devices=N_CORES)

    A1 = nc.dram_tensor("a1", [128, KC, S1], FP8, kind="ExternalInput")
    W1M = nc.dram_tensor("w1m", [128, HID_M // 128, KC, 128], FP8, kind="ExternalInput")
    W2M = nc.dram_tensor("w2m", [OUT_M, HID_M // 128, 128], FP8, kind="ExternalInput")
    B1M = nc.dram_tensor("b1m", [128, HID_M // 128], F32, kind="ExternalInput")
    B2M = nc.dram_tensor("b2m", [128, OUT_M // 128], F32, kind="ExternalInput")
    A23 = nc.dram_tensor("a23", [128, KC, PXF], BF16, kind="ExternalInput")
    W1F = nc.dram_tensor("w1f", [128, HID_F // 128, KC, 128], BF16, kind="ExternalInput")
    W2F = nc.dram_tensor("w2f", [OUT_F, HID_F // 128, 128], BF16, kind="ExternalInput")
    B1F = nc.dram_tensor("b1f", [128, HID_F // 128], F32, kind="ExternalInput")
    B2F = nc.dram_tensor("b2f", [128, OUT_F // 128], F32, kind="ExternalInput")

    O1 = nc.dram_tensor("o1", [OUT_M, S1], BF16, kind="ExternalOutput")
    O2 = nc.dram_tensor("o2", [OUT_F, PXF], BF16, kind="ExternalOutput")

    HM = HID_M // 128   # 96
    OM = OUT_M // 128   # 48
    HF = HID_F // 128   # 24
    OF = OUT_F // 128   # 12

    with tile.TileContext(nc) as tc:
        # ---------- M pipeline: fp8 DoubleRow, 2 pixel halves of 1024 ----------
        with ExitStack() as mctx:
            cp = mctx.enter_context(tc.tile_pool(name="m_const", bufs=1))
            w1p = mctx.enter_context(tc.tile_pool(name="m_w1", bufs=2))
            w2p = mctx.enter_context(tc.tile_pool(name="m_w2", bufs=2))
            h1p = mctx.enter_context(tc.tile_pool(name="m_h1", bufs=1))
            op = mctx.enter_context(tc.tile_pool(name="m_out", bufs=4))
            pp = mctx.enter_context(tc.tile_pool(name="m_ps", bufs=8, space="PSUM"))

            a1t = cp.tile([128, KC, S1], FP8)
            nc.sync.dma_start(a1t[:], A1[:])
            b1t = cp.tile([128, HM], F32)
            nc.sync.dma_start(b1t[:], B1M[:])
            b2t = cp.tile([128, OM], F32)
            nc.sync.dma_start(b2t[:], B2M[:])

            for hf in range(2):
                h1t = h1p.tile([128, HM, 1024], FP8, tag="h1")
                # conv1: 96 hid strips in groups of 8
                for g in range(12):
                    w1t = w1p.tile([128, 8, KC, 128], FP8, tag="w1")
                    nc.sync.dma_start(w1t[:], W1M[:, bass.ds(g * 8, 8), :, :])
                    for s in range(8):
                        i = g * 8 + s
                        for sb in range(2):
                            ps = pp.tile([128, 512], F32, tag="ps")
                            for c in range(3):
                                nc.tensor.matmul(
                                    ps[:],
                                    w1t[:, s, bass.ds(2 * c, 2), :],
                                    a1t[:, bass.ds(2 * c, 2), bass.ds(hf * 1024 + sb * 512, 512)],
                                    start=(c == 0), stop=(c == 2),
                                    perf_mode=DR,
                                )
                            nc.scalar.activation(
                                h1t[:, i, bass.ds(sb * 512, 512)], ps[:], RELU,
                                bias=b1t[:, i:i + 1],
                            )
                # conv2: 48 out strips, stream weights
                for o in range(OM):
                    w2t = w2p.tile([128, HM, 128], FP8, tag="w2")
                    nc.sync.dma_start(w2t[:], W2M[bass.ds(o * 128, 128), :, :])
                    for sb in range(2):
                        ps = pp.tile([128, 512], F32, tag="ps")
                        for j in range(48):
                            nc.tensor.matmul(
                                ps[:],
                                w2t[:, bass.ds(2 * j, 2), :],
                                h1t[:, bass.ds(2 * j, 2), bass.ds(sb * 512, 512)],
                                start=(j == 0), stop=(j == 47),
                                perf_mode=DR,
                            )
                        ot = op.tile([128, 512], BF16, tag="ot")
                        nc.scalar.activation(ot[:], ps[:], RELU, bias=b2t[:, o:o + 1])
                        nc.sync.dma_start(
                            O1[bass.ds(o * 128, 128), bass.ds(hf * 1024 + sb * 512, 512)],
                            ot[:],
                        )

        # ---------- F pipeline: bf16, re/im merged along pixels ----------
        with ExitStack() as fctx:
            cfp = fctx.enter_context(tc.tile_pool(name="f_const", bufs=1))
            w2fp = fctx.enter_context(tc.tile_pool(name="f_w2", bufs=2))
            h1fp = fctx.enter_context(tc.tile_pool(name="f_h1", bufs=1))
            ofp = fctx.enter_context(tc.tile_pool(name="f_out", bufs=4))
            fpp = fctx.enter_context(tc.tile_pool(name="f_ps", bufs=8, space="PSUM"))

            a23t = cfp.tile([128, KC, PXF], BF16)
            nc.sync.dma_start(a23t[:], A23[:])
            w1ft = cfp.tile([128, HF, KC, 128], BF16)
            nc.sync.dma_start(w1ft[:], W1F[:])
            fb1t = cfp.tile([128, HF], F32)
            nc.sync.dma_start(fb1t[:], B1F[:])
            fb2t = cfp.tile([128, OF], F32)
            nc.sync.dma_start(fb2t[:], B2F[:])

            h1ft = h1fp.tile([128, HF, PXF], BF16)
            for i in range(HF):
                for nb in range(NBF):
                    ps = fpp.tile([128, BLKF], F32, tag="ps")
                    for c in range(KC):
                        nc.tensor.matmul(
                            ps[:],
                            w1ft[:, i, c, :],
                            a23t[:, c, bass.ds(nb * BLKF, BLKF)],
                            start=(c == 0), stop=(c == KC - 1),
                        )
                    nc.scalar.activation(
                        h1ft[:, i, bass.ds(nb * BLKF, BLKF)], ps[:], RELU,
                        bias=fb1t[:, i:i + 1],
                    )
            for o in range(OF):
                w2ft = w2fp.tile([128, HF, 128], BF16, tag="w2f")
                nc.sync.dma_start(w2ft[:], W2F[bass.ds(o * 128, 128), :, :])
                for nb in range(NBF):
                    ps = fpp.tile([128, BLKF], F32, tag="ps")
                    for j in range(HF):
                        nc.tensor.matmul(
                            ps[:],
                            w2ft[:, j, :],
                            h1ft[:, j, bass.ds(nb * BLKF, BLKF)],
                            start=(j == 0), stop=(j == HF - 1),
                        )
                    ot = ofp.tile([128, BLKF], BF16, tag="otf")
                    nc.scalar.activation(ot[:], ps[:], RELU, bias=fb2t[:, o:o + 1])
                    nc.sync.dma_start(
                        O2[bass.ds(o * 128, 128), bass.ds(nb * BLKF, BLKF)], ot[:]
                    )

    nc.compile()
    _PROGRAM = nc
    return nc


def _fp8(x):
    return np.clip(np.ascontiguousarray(x), -240, 240).astype(ml_dtypes.float8_e4m3)


def _bf16(x):
    return np.ascontiguousarray(x).astype(ml_dtypes.bfloat16)


def kernel(x, mod_embed, norm1_w, norm1_b, norm2_w, norm2_b, w1, b1, w2, b2,
           f_c1_w, f_c1_b, f_c2_w, f_c2_b, fc1_w, fc1_b, fc2_w, fc2_b,
           m_c1_w, m_c1_b, m_c2_w, m_c2_b):
    x = np.asarray(x, np.float32)
    mod_embed = np.asarray(mod_embed, np.float32)
    B = x.shape[0]
    assert B == 1 and x.shape == (1, H, W, EMBED)

    # ---- host: LN1 + forward FFTs (cheap) ----
    residual = x
    xn = _layernorm(x, np.asarray(norm1_w, np.float32), np.asarray(norm1_b, np.float32))
    xf = np.fft.rfft2(xn[0].astype(np.float64), axes=(0, 1), norm="ortho")  # [H, WF, C]
    mf = np.fft.rfft2(np.asarray(mod_embed[0], np.float64), axes=(0, 1), norm="ortho")
    mr_f = np.ascontiguousarray(mf.real.astype(np.float32)).reshape(SPEC_TOT, EMBED)
    mi_f = np.ascontiguousarray(mf.imag.astype(np.float32)).reshape(SPEC_TOT, EMBED)

    nc = _build_program()

    HM = HID_M // 128
    OM = OUT_M // 128
    HF = HID_F // 128
    OF = OUT_F // 128

    # weights: partition-major packing so every device DMA is contiguous
    w1m_h = _fp8(np.asarray(m_c1_w, np.float32).reshape(HM, 128, KC, 128).transpose(3, 0, 2, 1))
    w2m_h = _fp8(np.asarray(m_c2_w, np.float32).reshape(OM, 128, HM, 128)
                 .transpose(0, 3, 2, 1).reshape(OUT_M, HM, 128))
    w1f_h = _bf16(np.asarray(f_c1_w, np.float32).reshape(HF, 128, KC, 128).transpose(3, 0, 2, 1))
    w2f_h = _bf16(np.asarray(f_c2_w, np.float32).reshape(OF, 128, HF, 128)
                  .transpose(0, 3, 2, 1).reshape(OUT_F, HF, 128))
    shared = {
        "w1m": w1m_h, "b1m": np.asarray(m_c1_b, np.float32).reshape(HM, 128).T.copy(),
        "w2m": w2m_h, "b2m": np.asarray(m_c2_b, np.float32).reshape(OM, 128).T.copy(),
        "w1f": w1f_h, "b1f": np.asarray(f_c1_b, np.float32).reshape(HF, 128).T.copy(),
        "w2f": w2f_h, "b2f": np.asarray(f_c2_b, np.float32).reshape(OF, 128).T.copy(),
    }

    modp = mod_embed[0].reshape(H * W, EMBED)
    in_maps = []
    for k in range(N_CORES):
        m = dict(shared)
        a1 = modp[k * S1:(k + 1) * S1].T.reshape(KC, 128, S1).transpose(1, 0, 2)
        m["a1"] = _fp8(a1)
        cat = np.concatenate(
            [mr_f[k * S2:(k + 1) * S2], mi_f[k * S2:(k + 1) * S2]], 0
        )  # [PXF, EMBED]
        a23 = cat.T.reshape(KC, 128, PXF).transpose(1, 0, 2)
        m["a23"] = _bf16(a23)
        in_maps.append(m)

    res = run_bass_kernel_spmd(nc, in_maps, core_ids=list(range(N_CORES)))
    global LAST_RESULT
    LAST_RESULT = res

    # reassemble (device already applied final ReLU)
    ss_mlp = np.concatenate(
        [res.results[k]["o1"].astype(np.float32).T for k in range(N_CORES)], 0
    )  # [16384, 6144]
    fo = [res.results[k]["o2"].astype(np.float32) for k in range(N_CORES)]
    fo_re = np.concatenate([f[:, :S2].T for f in fo], 0)   # [8320, 1536]
    fo_im = np.concatenate([f[:, S2:].T for f in fo], 0)

    # ---- host: rest of the filter ----
    xr = xf.real.astype(np.float32).reshape(1, H, WF, BLOCKS, BS)
    xi = xf.imag.astype(np.float32).reshape(1, H, WF, BLOCKS, BS)
    w1_ = np.asarray(w1, np.float32)
    b1_ = np.asarray(b1, np.float32)
    w2_ = np.asarray(w2, np.float32)
    b2_ = np.asarray(b2, np.float32)
    o1_re = _blockmm(xr, w1_[0]) - _blockmm(xi, w1_[1]) + b1_[0]
    o1_im = _blockmm(xi, w1_[0]) + _blockmm(xr, w1_[1]) + b1_[1]

    sc_re = 1.0 + fo_re[:, :EMBED].reshape(1, H, WF, BLOCKS, BS)
    sh_re = fo_re[:, EMBED:].reshape(1, H, WF, BLOCKS, BS)
    sc_im = 1.0 + fo_im[:, :EMBED].reshape(1, H, WF, BLOCKS, BS)
    sh_im = fo_im[:, EMBED:].reshape(1, H, WF, BLOCKS, BS)

    n_re = o1_re * sc_re - o1_im * sc_im + sh_re
    n_im = o1_im * sc_re + o1_re * sc_im + sh_im
    o1_re = np.maximum(n_re, 0.0)
    o1_im = np.maximum(n_im, 0.0)

    o2_re = _softshrink(_blockmm(o1_re, w2_[0]) - _blockmm(o1_im, w2_[1]) + b2_[0], LAMBD)
    o2_im = _softshrink(_blockmm(o1_im, w2_[0]) + _blockmm(o1_re, w2_[1]) + b2_[1], LAMBD)

    spec = (o2_re + 1j * o2_im).reshape(H, WF, EMBED)
    filt = np.fft.irfft2(spec, s=(H, W), axes=(0, 1), norm="ortho").astype(np.float32)
    h_mid = filt[None] + xn + residual  # filter bias (xn) + double_skip residual

    # ---- host: second half (device did scale/shift) ----
    h2 = _layernorm(h_mid, np.asarray(norm2_w, np.float32), np.asarray(norm2_b, np.float32))
    scale = 1.0 + ss_mlp[:, :LATENT].reshape(1, H, W, LATENT)
    shift = ss_mlp[:, LATENT:].reshape(1, H, W, LATENT)
    hh = h2.reshape(H * W, EMBED) @ np.asarray(fc1_w, np.float32).T + np.asarray(fc1_b, np.float32)
    hh = hh.reshape(1, H, W, LATENT) * scale + shift
    hh = _gelu(hh)
    out = hh.reshape(H * W, LATENT) @ np.asarray(fc2_w, np.float32).T + np.asarray(fc2_b, np.float32)
    return (out.reshape(1, H, W, EMBED) + h_mid).astype(np.float32)


# revision 13
# speedup vs baseline: 3.5365x; 1.0300x over previous
import sys

sys.path.insert(0, "/opt/trn_rl_repo")
import numpy as np
import ml_dtypes

import concourse.bass as bass
import concourse.tile as tile
import concourse.bacc as bacc
from concourse import mybir
from concourse.bass_utils import run_bass_kernel_spmd

# bass_utils' axon trace path hard-imports antenv.axon_hooks; provide a
# null-hook shim when the image lacks it so tracing degrades gracefully
# instead of crashing kernel().
try:
    import antenv.axon_hooks  # noqa: F401
except ImportError:
    import types as _types

    _hook_store = {"fn": None}
    _m = _types.ModuleType("antenv.axon_hooks")
    _m.set_axon_ntff_profile_hook = lambda h: _hook_store.__setitem__("fn", h)
    _m.get_axon_ntff_profile_hook = lambda: _hook_store["fn"]
    sys.modules["antenv.axon_hooks"] = _m

BF16 = mybir.dt.bfloat16
F32 = mybir.dt.float32
FP8 = mybir.dt.float8e4
DR = mybir.MatmulPerfMode.DoubleRow
RELU = mybir.ActivationFunctionType.Relu

N_CORES = 8
EMBED = 768
KC = 6            # 768 / 128 contraction chunks
BLOCKS = 8
BS = 96
LATENT = 4 * EMBED            # 3072
HID_M = 4 * LATENT            # 12288
OUT_M = 2 * LATENT            # 6144
HID_F = 4 * EMBED             # 3072
OUT_F = 2 * EMBED             # 1536
LAMBD = 0.01
EPS = 1e-5
H = 128
W = 128
WF = 65
SPEC_TOT = H * WF             # 8320
S1 = (H * W) // N_CORES       # 2048 spatial px per core
S2 = SPEC_TOT // N_CORES      # 1040 spectral px per core
PXF = 2 * S2                  # 2080 (re ++ im)
NBF = 5
BLKF = PXF // NBF             # 416 (psum-bank safe)


def _erf(x):
    a1, a2, a3, a4, a5, p = (
        0.254829592, -0.284496736, 1.421413741, -1.453152027, 1.061405429, 0.3275911,
    )
    s = np.sign(x)
    ax = np.abs(x)
    t = 1.0 / (1.0 + p * ax)
    y = 1.0 - (((((a5 * t + a4) * t) + a3) * t + a2) * t + a1) * t * np.exp(-ax * ax)
    return s * y


def _gelu(x):
    return 0.5 * x * (1.0 + _erf(x / np.sqrt(2.0)))


def _layernorm(x, w, b):
    m = x.mean(-1, keepdims=True)
    v = x.var(-1, keepdims=True)
    return (x - m) / np.sqrt(v + EPS) * w + b


def _softshrink(x, l):
    return np.where(x > l, x - l, np.where(x < -l, x + l, 0.0)).astype(np.float32)


def _blockmm(x, w):
    return np.einsum("nyxbi,bio->nyxbo", x, w, optimize=True)


_PROGRAM = None
LAST_RESULT = None


def _build_program():
    global _PROGRAM
    if _PROGRAM is not None:
        return _PROGRAM
    from contextlib import ExitStack

    nc = bacc.Bacc("TRN2", target_bir_lowering=False, debug=False, num_devices=N_CORES)

    A1 = nc.dram_tensor("a1", [128, KC, S1], FP8, kind="ExternalInput")
    W1M = nc.dram_tensor("w1m", [128, HID_M // 128, KC, 128], FP8, kind="ExternalInput")
    W2M = nc.dram_tensor("w2m", [OUT_M, HID_M // 128, 128], FP8, kind="ExternalInput")
    B1M = nc.dram_tensor("b1m", [128, HID_M // 128], F32, kind="ExternalInput")
    B2M = nc.dram_tensor("b2m", [128, OUT_M // 128], F32, kind="ExternalInput")
    A23 = nc.dram_tensor("a23", [128, KC, PXF], FP8, kind="ExternalInput")
    W1F = nc.dram_tensor("w1f", [128, HID_F // 128, KC, 128], FP8, kind="ExternalInput")
    W2F = nc.dram_tensor("w2f", [OUT_F, HID_F // 128, 128], BF16, kind="ExternalInput")
    B1F = nc.dram_tensor("b1f", [128, HID_F // 128], F32, kind="ExternalInput")
    B2F = nc.dram_tensor("b2f", [128, OUT_F // 128], F32, kind="ExternalInput")

    O1 = nc.dram_tensor("o1", [OUT_M, S1], BF16, kind="ExternalOutput")
    O2 = nc.dram_tensor("o2", [OUT_F, PXF], BF16, kind="ExternalOutput")

    HM = HID_M // 128   # 96
    OM = OUT_M // 128   # 48
    HF = HID_F // 128   # 24
    OF = OUT_F // 128   # 12

    with tile.TileContext(nc) as tc, ExitStack() as octx:
        # ---------- M pipeline: fp8 DoubleRow, 2 pixel halves of 1024 ----------
        with ExitStack() as mctx:
            cp = mctx.enter_context(tc.tile_pool(name="m_const", bufs=1))
            w1p = mctx.enter_context(tc.tile_pool(name="m_w1", bufs=2))
            w2p = mctx.enter_context(tc.tile_pool(name="m_w2", bufs=2))
            h1p = mctx.enter_context(tc.tile_pool(name="m_h1", bufs=1))
            op = mctx.enter_context(tc.tile_pool(name="m_out", bufs=4))
            pp = mctx.enter_context(tc.tile_pool(name="m_ps", bufs=8, space="PSUM"))

            a1t = cp.tile([128, KC, S1], FP8)
            nc.sync.dma_start(a1t[:], A1[:])
            b1t = cp.tile([128, HM], F32)
            nc.sync.dma_start(b1t[:], B1M[:])
            b2t = cp.tile([128, OM], F32)
            nc.sync.dma_start(b2t[:], B2M[:])

            for hf in range(2):
                h1t = h1p.tile([128, HM, 1024], FP8, tag="h1")
                # conv1: 96 hid strips in groups of 8
                for g in range(12):
                    w1t = w1p.tile([128, 8, KC, 128], FP8, tag="w1")
                    nc.sync.dma_start(w1t[:], W1M[:, bass.ds(g * 8, 8), :, :])
                    for s in range(8):
                        i = g * 8 + s
                        for sb in range(2):
                            ps = pp.tile([128, 512], F32, tag="ps")
                            for c in range(3):
                                nc.tensor.matmul(
                                    ps[:],
                                    w1t[:, s, bass.ds(2 * c, 2), :],
                                    a1t[:, bass.ds(2 * c, 2), bass.ds(hf * 1024 + sb * 512, 512)],
                                    start=(c == 0), stop=(c == 2),
                                    perf_mode=DR,
                                )
                            nc.scalar.activation(
                                h1t[:, i, bass.ds(sb * 512, 512)], ps[:], RELU,
                                bias=b1t[:, i:i + 1],
                            )
                # conv2: 48 out strips, stream weights
                for o in range(OM):
                    w2t = w2p.tile([128, HM, 128], FP8, tag="w2")
                    nc.sync.dma_start(w2t[:], W2M[bass.ds(o * 128, 128), :, :])
                    for sb in range(2):
                        ps = pp.tile([128, 512], F32, tag="ps")
                        for j in range(48):
                            nc.tensor.matmul(
                                ps[:],
                                w2t[:, bass.ds(2 * j, 2), :],
                                h1t[:, bass.ds(2 * j, 2), bass.ds(sb * 512, 512)],
                                start=(j == 0), stop=(j == 47),
                                perf_mode=DR,
                            )
                        ot = op.tile([128, 512], BF16, tag="ot")
                        nc.scalar.activation(ot[:], ps[:], RELU, bias=b2t[:, o:o + 1])
                        nc.sync.dma_start(
                            O1[bass.ds(o * 128, 128), bass.ds(hf * 1024 + sb * 512, 512)],
                            ot[:],
                        )

        # ---------- F pipeline: fp8 conv1 (DoubleRow) + bf16 conv2 ----------
        with ExitStack() as fctx:
            cfp = fctx.enter_context(tc.tile_pool(name="f_const", bufs=1))
            w2fp = fctx.enter_context(tc.tile_pool(name="f_w2", bufs=2))
            h1fp = fctx.enter_context(tc.tile_pool(name="f_h1", bufs=1))
            ofp = fctx.enter_context(tc.tile_pool(name="f_out", bufs=4))
            fpp = fctx.enter_context(tc.tile_pool(name="f_ps", bufs=8, space="PSUM"))

            a23t = cfp.tile([128, KC, PXF], FP8)
            nc.sync.dma_start(a23t[:], A23[:])
            w1ft = cfp.tile([128, HF, KC, 128], FP8)
            nc.sync.dma_start(w1ft[:], W1F[:])
            fb1t = cfp.tile([128, HF], F32)
            nc.sync.dma_start(fb1t[:], B1F[:])
            fb2t = cfp.tile([128, OF], F32)
            nc.sync.dma_start(fb2t[:], B2F[:])

            h1ft = h1fp.tile([128, HF, PXF], BF16)
            for i in range(HF):
                for nb in range(NBF):
                    ps = fpp.tile([128, BLKF], F32, tag="ps")
                    for c in range(KC // 2):
                        nc.tensor.matmul(
                            ps[:],
                            w1ft[:, i, bass.ds(2 * c, 2), :],
                            a23t[:, bass.ds(2 * c, 2), bass.ds(nb * BLKF, BLKF)],
                            start=(c == 0), stop=(c == KC // 2 - 1),
                            perf_mode=DR,
                        )
                    nc.scalar.activation(
                        h1ft[:, i, bass.ds(nb * BLKF, BLKF)], ps[:], RELU,
                        bias=fb1t[:, i:i + 1],
                    )
            for o in range(OF):
                w2ft = w2fp.tile([128, HF, 128], BF16, tag="w2f")
                nc.sync.dma_start(w2ft[:], W2F[bass.ds(o * 128, 128), :, :])
                for nb in range(NBF):
                    ps = fpp.tile([128, BLKF], F32, tag="ps")
                    for j in range(HF):
                        nc.tensor.matmul(
                            ps[:],
                            w2ft[:, j, :],
                            h1ft[:, j, bass.ds(nb * BLKF, BLKF)],
                            start=(j == 0), stop=(j == HF - 1),
                        )
                    ot = ofp.tile([128, BLKF], BF16, tag="otf")
                    nc.scalar.activation(ot[:], ps[:], RELU, bias=fb2t[:, o:o + 1])
                    nc.sync.dma_start(
                        O2[bass.ds(o * 128, 128), bass.ds(nb * BLKF, BLKF)], ot[:]
                    )

    nc.compile()
    _PROGRAM = nc
    return nc


def _fp8(x):
    return np.clip(np.ascontiguousarray(x), -240, 240).astype(ml_dtypes.float8_e4m3)


def _bf16(x):
    return np.ascontiguousarray(x).astype(ml_dtypes.bfloat16)


def kernel(x, mod_embed, norm1_w, norm1_b, norm2_w, norm2_b, w1, b1, w2, b2,
           f_c1_w, f_c1_b, f_c2_w, f_c2_b, fc1_w, fc1_b, fc2_w, fc2_b,
           m_c1_w, m_c1_b, m_c2_w, m_c2_b):
    x = np.asarray(x, np.float32)
    mod_embed = np.asarray(mod_embed, np.float32)
    B = x.shape[0]
    assert B == 1 and x.shape == (1, H, W, EMBED)

    # ---- host: LN1 + forward FFTs (cheap) ----
    residual = x
    xn = _layernorm(x, np.asarray(norm1_w, np.float32), np.asarray(norm1_b, np.float32))
    xf = np.fft.rfft2(xn[0].astype(np.float64), axes=(0, 1), norm="ortho")  # [H, WF, C]
    mf = np.fft.rfft2(np.asarray(mod_embed[0], np.float64), axes=(0, 1), norm="ortho")
    mr_f = np.ascontiguousarray(mf.real.astype(np.float32)).reshape(SPEC_TOT, EMBED)
    mi_f = np.ascontiguousarray(mf.imag.astype(np.float32)).reshape(SPEC_TOT, EMBED)

    nc = _build_program()

    HM = HID_M // 128
    OM = OUT_M // 128
    HF = HID_F // 128
    OF = OUT_F // 128

    # weights: partition-major packing so every device DMA is contiguous
    w1m_h = _fp8(np.asarray(m_c1_w, np.float32).reshape(HM, 128, KC, 128).transpose(3, 0, 2, 1))
    w2m_h = _fp8(np.asarray(m_c2_w, np.float32).reshape(OM, 128, HM, 128)
                 .transpose(0, 3, 2, 1).reshape(OUT_M, HM, 128))
    w1f_h = _fp8(np.asarray(f_c1_w, np.float32).reshape(HF, 128, KC, 128).transpose(3, 0, 2, 1))
    w2f_h = _bf16(np.asarray(f_c2_w, np.float32).reshape(OF, 128, HF, 128)
                  .transpose(0, 3, 2, 1).reshape(OUT_F, HF, 128))
    shared = {
        "w1m": w1m_h, "b1m": np.asarray(m_c1_b, np.float32).reshape(HM, 128).T.copy(),
        "w2m": w2m_h, "b2m": np.asarray(m_c2_b, np.float32).reshape(OM, 128).T.copy(),
        "w1f": w1f_h, "b1f": np.asarray(f_c1_b, np.float32).reshape(HF, 128).T.copy(),
        "w2f": w2f_h, "b2f": np.asarray(f_c2_b, np.float32).reshape(OF, 128).T.copy(),
    }

    modp = mod_embed[0].reshape(H * W, EMBED)
    in_maps = []
    for k in range(N_CORES):
        m = dict(shared)
        a1 = modp[k * S1:(k + 1) * S1].T.reshape(KC, 128, S1).transpose(1, 0, 2)
        m["a1"] = _fp8(a1)
        cat = np.concatenate(
            [mr_f[k * S2:(k + 1) * S2], mi_f[k * S2:(k + 1) * S2]], 0
        )  # [PXF, EMBED]
        a23 = cat.T.reshape(KC, 128, PXF).transpose(1, 0, 2)
        m["a23"] = _fp8(a23)
        in_maps.append(m)

    res = run_bass_kernel_spmd(nc, in_maps, core_ids=list(range(N_CORES)))
    global LAST_RESULT
    LAST_RESULT = res

    # reassemble (device already applied final ReLU)
    ss_mlp = np.concatenate(
        [res.results[k]["o1"].astype(np.float32).T for k in range(N_CORES)], 0
    )  # [16384, 6144]
    fo = [res.results[k]["o2"].astype(np.float32) for k in range(N_CORES)]
    fo_re = np.concatenate([f[:, :S2].T for f in fo], 0)   # [8320, 1536]
    fo_im = np.concatenate([f[:, S2:].T for f in fo], 0)

    # ---- host: rest of the filter ----
    xr = xf.real.astype(np.float32).reshape(1, H, WF, BLOCKS, BS)
    xi = xf.imag.astype(np.float32).reshape(1, H, WF, BLOCKS, BS)
    w1_ = np.asarray(w1, np.float32)
    b1_ = np.asarray(b1, np.float32)
    w2_ = np.asarray(w2, np.float32)
    b2_ = np.asarray(b2, np.float32)
    o1_re = _blockmm(xr, w1_[0]) - _blockmm(xi, w1_[1]) + b1_[0]
    o1_im = _blockmm(xi, w1_[0]) + _blockmm(xr, w1_[1]) + b1_[1]

    sc_re = 1.0 + fo_re[:, :EMBED].reshape(1, H, WF, BLOCKS, BS)
    sh_re = fo_re[:, EMBED:].reshape(1, H, WF, BLOCKS, BS)
    sc_im = 1.0 + fo_im[:, :EMBED].reshape(1, H, WF, BLOCKS, BS)
    sh_im = fo_im[:, EMBED:].reshape(1, H, WF, BLOCKS, BS)

    n_re = o1_re * sc_re - o1_im * sc_im + sh_re
    n_im = o1_im * sc_re + o1_re * sc_im + sh_im
    o1_re = np.maximum(n_re, 0.0)
    o1_im = np.maximum(n_im, 0.0)

    o2_re = _softshrink(_blockmm(o1_re, w2_[0]) - _blockmm(o1_im, w2_[1]) + b2_[0], LAMBD)
    o2_im = _softshrink(_blockmm(o1_im, w2_[0]) + _blockmm(o1_re, w2_[1]) + b2_[1], LAMBD)

    spec = (o2_re + 1j * o2_im).reshape(H, WF, EMBED)
    filt = np.fft.irfft2(spec, s=(H, W), axes=(0, 1), norm="ortho").astype(np.float32)
    h_mid = filt[None] + xn + residual  # filter bias (xn) + double_skip residual

    # ---- host: second half (device did scale/shift) ----
    h2 = _layernorm(h_mid, np.asarray(norm2_w, np.float32), np.asarray(norm2_b, np.float32))
    scale = 1.0 + ss_mlp[:, :LATENT].reshape(1, H, W, LATENT)
    shift = ss_mlp[:, LATENT:].reshape(1, H, W, LATENT)
    hh = h2.reshape(H * W, EMBED) @ np.asarray(fc1_w, np.float32).T + np.asarray(fc1_b, np.float32)
    hh = hh.reshape(1, H, W, LATENT) * scale + shift
    hh = _gelu(hh)
    out = hh.reshape(H * W, LATENT) @ np.asarray(fc2_w, np.float32).T + np.asarray(fc2_b, np.float32)
    return (out.reshape(1, H, W, EMBED) + h_mid).astype(np.float32)


# revision 17
# speedup vs baseline: 3.5555x; 1.0054x over previous
import sys

sys.path.insert(0, "/opt/trn_rl_repo")
import numpy as np
import ml_dtypes

import concourse.bass as bass
import concourse.tile as tile
import concourse.bacc as bacc
from concourse import mybir
from concourse.bass_utils import run_bass_kernel_spmd

# bass_utils' axon trace path hard-imports antenv.axon_hooks; provide a
# null-hook shim when the image lacks it so tracing degrades gracefully
# instead of crashing kernel().
try:
    import antenv.axon_hooks  # noqa: F401
except ImportError:
    import types as _types

    _hook_store = {"fn": None}
    _m = _types.ModuleType("antenv.axon_hooks")
    _m.set_axon_ntff_profile_hook = lambda h: _hook_store.__setitem__("fn", h)
    _m.get_axon_ntff_profile_hook = lambda: _hook_store["fn"]
    sys.modules["antenv.axon_hooks"] = _m

BF16 = mybir.dt.bfloat16
F32 = mybir.dt.float32
FP8 = mybir.dt.float8e4
DR = mybir.MatmulPerfMode.DoubleRow
RELU = mybir.ActivationFunctionType.Relu

N_CORES = 8
EMBED = 768
KC = 6            # 768 / 128 contraction chunks
BLOCKS = 8
BS = 96
LATENT = 4 * EMBED            # 3072
HID_M = 4 * LATENT            # 12288
OUT_M = 2 * LATENT            # 6144
HID_F = 4 * EMBED             # 3072
OUT_F = 2 * EMBED             # 1536
LAMBD = 0.01
EPS = 1e-5
H = 128
W = 128
WF = 65
SPEC_TOT = H * WF             # 8320
S1 = (H * W) // N_CORES       # 2048 spatial px per core
S2 = SPEC_TOT // N_CORES      # 1040 spectral px per core
PXF = 2 * S2                  # 2080 (re ++ im)
NBF = 5
BLKF = PXF // NBF             # 416 (psum-bank safe)


def _erf(x):
    a1, a2, a3, a4, a5, p = (
        0.254829592, -0.284496736, 1.421413741, -1.453152027, 1.061405429, 0.3275911,
    )
    s = np.sign(x)
    ax = np.abs(x)
    t = 1.0 / (1.0 + p * ax)
    y = 1.0 - (((((a5 * t + a4) * t) + a3) * t + a2) * t + a1) * t * np.exp(-ax * ax)
    return s * y


def _gelu(x):
    return 0.5 * x * (1.0 + _erf(x / np.sqrt(2.0)))


def _layernorm(x, w, b):
    m = x.mean(-1, keepdims=True)
    v = x.var(-1, keepdims=True)
    return (x - m) / np.sqrt(v + EPS) * w + b


def _softshrink(x, l):
    return np.where(x > l, x - l, np.where(x < -l, x + l, 0.0)).astype(np.float32)


def _blockmm(x, w):
    return np.einsum("nyxbi,bio->nyxbo", x, w, optimize=True)


_PROGRAM = None
LAST_RESULT = None


def _build_program():
    global _PROGRAM
    if _PROGRAM is not None:
        return _PROGRAM
    from contextlib import ExitStack

    nc = bacc.Bacc("TRN2", target_bir_lowering=False, debug=False, num_devices=N_CORES)

    A1 = nc.dram_tensor("a1", [128, KC, S1], FP8, kind="ExternalInput")
    W1M = nc.dram_tensor("w1m", [128, HID_M // 128, KC, 128], FP8, kind="ExternalInput")
    W2M = nc.dram_tensor("w2m", [OUT_M, HID_M // 128, 128], FP8, kind="ExternalInput")
    B1M = nc.dram_tensor("b1m", [128, HID_M // 128], F32, kind="ExternalInput")
    B2M = nc.dram_tensor("b2m", [128, OUT_M // 128], F32, kind="ExternalInput")
    A23 = nc.dram_tensor("a23", [128, KC, PXF], FP8, kind="ExternalInput")
    W1F = nc.dram_tensor("w1f", [128, HID_F // 128, KC, 128], FP8, kind="ExternalInput")
    W2F8 = nc.dram_tensor("w2f8", [OUT_F // 2, HID_F // 128, 128], FP8, kind="ExternalInput")
    W2FB = nc.dram_tensor("w2fb", [OUT_F // 2, HID_F // 128, 128], BF16, kind="ExternalInput")
    B1F = nc.dram_tensor("b1f", [128, HID_F // 128], F32, kind="ExternalInput")
    B2F = nc.dram_tensor("b2f", [128, OUT_F // 128], F32, kind="ExternalInput")

    O1 = nc.dram_tensor("o1", [OUT_M, S1], BF16, kind="ExternalOutput")
    O2 = nc.dram_tensor("o2", [OUT_F, PXF], BF16, kind="ExternalOutput")

    HM = HID_M // 128   # 96
    OM = OUT_M // 128   # 48
    HF = HID_F // 128   # 24
    OF = OUT_F // 128   # 12

    with tile.TileContext(nc) as tc, ExitStack() as octx:
        # ---------- M pipeline: fp8 DoubleRow, 2 pixel halves of 1024 ----------
        with ExitStack() as mctx:
            cp = mctx.enter_context(tc.tile_pool(name="m_const", bufs=1))
            w1p = mctx.enter_context(tc.tile_pool(name="m_w1", bufs=2))
            w2p = mctx.enter_context(tc.tile_pool(name="m_w2", bufs=2))
            h1p = mctx.enter_context(tc.tile_pool(name="m_h1", bufs=1))
            op = mctx.enter_context(tc.tile_pool(name="m_out", bufs=4))
            pp = mctx.enter_context(tc.tile_pool(name="m_ps", bufs=8, space="PSUM"))

            a1t = cp.tile([128, KC, S1], FP8)
            nc.sync.dma_start(a1t[:], A1[:])
            b1t = cp.tile([128, HM], F32)
            nc.sync.dma_start(b1t[:], B1M[:])
            b2t = cp.tile([128, OM], F32)
            nc.sync.dma_start(b2t[:], B2M[:])

            for hf in range(2):
                h1t = h1p.tile([128, HM, 1024], FP8, tag="h1")
                # conv1: 96 hid strips in groups of 8
                for g in range(12):
                    w1t = w1p.tile([128, 8, KC, 128], FP8, tag="w1")
                    nc.sync.dma_start(w1t[:], W1M[:, bass.ds(g * 8, 8), :, :])
                    for s in range(8):
                        i = g * 8 + s
                        for sb in range(2):
                            ps = pp.tile([128, 512], F32, tag="ps")
                            for c in range(3):
                                nc.tensor.matmul(
                                    ps[:],
                                    w1t[:, s, bass.ds(2 * c, 2), :],
                                    a1t[:, bass.ds(2 * c, 2), bass.ds(hf * 1024 + sb * 512, 512)],
                                    start=(c == 0), stop=(c == 2),
                                    perf_mode=DR,
                                )
                            nc.scalar.activation(
                                h1t[:, i, bass.ds(sb * 512, 512)], ps[:], RELU,
                                bias=b1t[:, i:i + 1],
                            )
                # conv2: 48 out strips, stream weights
                for o in range(OM):
                    w2t = w2p.tile([128, HM, 128], FP8, tag="w2")
                    nc.sync.dma_start(w2t[:], W2M[bass.ds(o * 128, 128), :, :])
                    for sb in range(2):
                        ps = pp.tile([128, 512], F32, tag="ps")
                        for j in range(48):
                            nc.tensor.matmul(
                                ps[:],
                                w2t[:, bass.ds(2 * j, 2), :],
                                h1t[:, bass.ds(2 * j, 2), bass.ds(sb * 512, 512)],
                                start=(j == 0), stop=(j == 47),
                                perf_mode=DR,
                            )
                        ot = op.tile([128, 512], BF16, tag="ot")
                        nc.scalar.activation(ot[:], ps[:], RELU, bias=b2t[:, o:o + 1])
                        nc.sync.dma_start(
                            O1[bass.ds(o * 128, 128), bass.ds(hf * 1024 + sb * 512, 512)],
                            ot[:],
                        )

        # ---------- F pipeline: fp8 conv1 (DoubleRow) + bf16 conv2 ----------
        with ExitStack() as fctx:
            cfp = fctx.enter_context(tc.tile_pool(name="f_const", bufs=1))
            w2fp = fctx.enter_context(tc.tile_pool(name="f_w2", bufs=2))
            h1fp = fctx.enter_context(tc.tile_pool(name="f_h1", bufs=1))
            ofp = fctx.enter_context(tc.tile_pool(name="f_out", bufs=4))
            fpp = fctx.enter_context(tc.tile_pool(name="f_ps", bufs=8, space="PSUM"))

            a23t = cfp.tile([128, KC, PXF], FP8)
            nc.sync.dma_start(a23t[:], A23[:])
            w1ft = cfp.tile([128, HF, KC, 128], FP8)
            nc.sync.dma_start(w1ft[:], W1F[:])
            fb1t = cfp.tile([128, HF], F32)
            nc.sync.dma_start(fb1t[:], B1F[:])
            fb2t = cfp.tile([128, OF], F32)
            nc.sync.dma_start(fb2t[:], B2F[:])

            h1ft = h1fp.tile([128, HF, PXF], BF16)
            h1f8t = h1fp.tile([128, HF, PXF], FP8)
            for i in range(HF):
                for nb in range(NBF):
                    ps = fpp.tile([128, BLKF], F32, tag="ps")
                    for c in range(KC // 2):
                        nc.tensor.matmul(
                            ps[:],
                            w1ft[:, i, bass.ds(2 * c, 2), :],
                            a23t[:, bass.ds(2 * c, 2), bass.ds(nb * BLKF, BLKF)],
                            start=(c == 0), stop=(c == KC // 2 - 1),
                            perf_mode=DR,
                        )
                    nc.scalar.activation(
                        h1ft[:, i, bass.ds(nb * BLKF, BLKF)], ps[:], RELU,
                        bias=fb1t[:, i:i + 1],
                    )
                    nc.scalar.activation(
                        h1f8t[:, i, bass.ds(nb * BLKF, BLKF)], ps[:], RELU,
                        bias=fb1t[:, i:i + 1],
                    )
            # scale half (output rows 0:768): fp8 DoubleRow — the scale
            # multiplies the small-amplitude spectral signal, so its fp8
            # noise is strongly attenuated; shift half stays bf16.
            for o in range(OF // 2):
                w2ft = w2fp.tile([128, HF, 128], FP8, tag="w2f8")
                nc.sync.dma_start(w2ft[:], W2F8[bass.ds(o * 128, 128), :, :])
                for nb in range(NBF):
                    ps = fpp.tile([128, BLKF], F32, tag="ps")
                    for j in range(HF // 2):
                        nc.tensor.matmul(
                            ps[:],
                            w2ft[:, bass.ds(2 * j, 2), :],
                            h1f8t[:, bass.ds(2 * j, 2), bass.ds(nb * BLKF, BLKF)],
                            start=(j == 0), stop=(j == HF // 2 - 1),
                            perf_mode=DR,
                        )
                    ot = ofp.tile([128, BLKF], BF16, tag="otf")
                    nc.scalar.activation(ot[:], ps[:], RELU, bias=fb2t[:, o:o + 1])
                    nc.sync.dma_start(
                        O2[bass.ds(o * 128, 128), bass.ds(nb * BLKF, BLKF)], ot[:]
                    )
            for oo in range(OF // 2):
                o = OF // 2 + oo
                w2ft = w2fp.tile([128, HF, 128], BF16, tag="w2fb")
                nc.sync.dma_start(w2ft[:], W2FB[bass.ds(oo * 128, 128), :, :])
                for nb in range(NBF):
                    ps = fpp.tile([128, BLKF], F32, tag="ps")
                    for j in range(HF):
                        nc.tensor.matmul(
                            ps[:],
                            w2ft[:, j, :],
                            h1ft[:, j, bass.ds(nb * BLKF, BLKF)],
                            start=(j == 0), stop=(j == HF - 1),
                        )
                    ot = ofp.tile([128, BLKF], BF16, tag="otf")
                    nc.scalar.activation(ot[:], ps[:], RELU, bias=fb2t[:, o:o + 1])
                    nc.sync.dma_start(
                        O2[bass.ds(o * 128, 128), bass.ds(nb * BLKF, BLKF)], ot[:]
                    )

    nc.compile()
    _PROGRAM = nc
    return nc


def _fp8(x):
    return np.clip(np.ascontiguousarray(x), -240, 240).astype(ml_dtypes.float8_e4m3)


def _bf16(x):
    return np.ascontiguousarray(x).astype(ml_dtypes.bfloat16)


def kernel(x, mod_embed, norm1_w, norm1_b, norm2_w, norm2_b, w1, b1, w2, b2,
           f_c1_w, f_c1_b, f_c2_w, f_c2_b, fc1_w, fc1_b, fc2_w, fc2_b,
           m_c1_w, m_c1_b, m_c2_w, m_c2_b):
    x = np.asarray(x, np.float32)
    mod_embed = np.asarray(mod_embed, np.float32)
    B = x.shape[0]
    assert B == 1 and x.shape == (1, H, W, EMBED)

    # ---- host: LN1 + forward FFTs (cheap) ----
    residual = x
    xn = _layernorm(x, np.asarray(norm1_w, np.float32), np.asarray(norm1_b, np.float32))
    xf = np.fft.rfft2(xn[0].astype(np.float64), axes=(0, 1), norm="ortho")  # [H, WF, C]
    mf = np.fft.rfft2(np.asarray(mod_embed[0], np.float64), axes=(0, 1), norm="ortho")
    mr_f = np.ascontiguousarray(mf.real.astype(np.float32)).reshape(SPEC_TOT, EMBED)
    mi_f = np.ascontiguousarray(mf.imag.astype(np.float32)).reshape(SPEC_TOT, EMBED)

    nc = _build_program()

    HM = HID_M // 128
    OM = OUT_M // 128
    HF = HID_F // 128
    OF = OUT_F // 128

    # weights: partition-major packing so every device DMA is contiguous
    w1m_h = _fp8(np.asarray(m_c1_w, np.float32).reshape(HM, 128, KC, 128).transpose(3, 0, 2, 1))
    w2m_h = _fp8(np.asarray(m_c2_w, np.float32).reshape(OM, 128, HM, 128)
                 .transpose(0, 3, 2, 1).reshape(OUT_M, HM, 128))
    w1f_h = _fp8(np.asarray(f_c1_w, np.float32).reshape(HF, 128, KC, 128).transpose(3, 0, 2, 1))
    w2f_pack = (np.asarray(f_c2_w, np.float32).reshape(OF, 128, HF, 128)
                .transpose(0, 3, 2, 1).reshape(OUT_F, HF, 128))
    w2f8_h = _fp8(w2f_pack[:OUT_F // 2])
    w2fb_h = _bf16(w2f_pack[OUT_F // 2:])
    shared = {
        "w1m": w1m_h, "b1m": np.asarray(m_c1_b, np.float32).reshape(HM, 128).T.copy(),
        "w2m": w2m_h, "b2m": np.asarray(m_c2_b, np.float32).reshape(OM, 128).T.copy(),
        "w1f": w1f_h, "b1f": np.asarray(f_c1_b, np.float32).reshape(HF, 128).T.copy(),
        "w2f8": w2f8_h, "w2fb": w2fb_h,
        "b2f": np.asarray(f_c2_b, np.float32).reshape(OF, 128).T.copy(),
    }

    modp = mod_embed[0].reshape(H * W, EMBED)
    in_maps = []
    for k in range(N_CORES):
        m = dict(shared)
        a1 = modp[k * S1:(k + 1) * S1].T.reshape(KC, 128, S1).transpose(1, 0, 2)
        m["a1"] = _fp8(a1)
        cat = np.concatenate(
            [mr_f[k * S2:(k + 1) * S2], mi_f[k * S2:(k + 1) * S2]], 0
        )  # [PXF, EMBED]
        a23 = cat.T.reshape(KC, 128, PXF).transpose(1, 0, 2)
        m["a23"] = _fp8(a23)
        in_maps.append(m)

    res = run_bass_kernel_spmd(nc, in_maps, core_ids=list(range(N_CORES)))
    global LAST_RESULT
    LAST_RESULT = res

    # reassemble (device already applied final ReLU)
    ss_mlp = np.concatenate(
        [res.results[k]["o1"].astype(np.float32).T for k in range(N_CORES)], 0
    )  # [16384, 6144]
    fo = [res.results[k]["o2"].astype(np.float32) for k in range(N_CORES)]
    fo_re = np.concatenate([f[:, :S2].T for f in fo], 0)   # [8320, 1536]
    fo_im = np.concatenate([f[:, S2:].T for f in fo], 0)

    # ---- host: rest of the filter ----
    xr = xf.real.astype(np.float32).reshape(1, H, WF, BLOCKS, BS)
    xi = xf.imag.astype(np.float32).reshape(1, H, WF, BLOCKS, BS)
    w1_ = np.asarray(w1, np.float32)
    b1_ = np.asarray(b1, np.float32)
    w2_ = np.asarray(w2, np.float32)
    b2_ = np.asarray(b2, np.float32)
    o1_re = _blockmm(xr, w1_[0]) - _blockmm(xi, w1_[1]) + b1_[0]
    o1_im = _blockmm(xi, w1_[0]) + _blockmm(xr, w1_[1]) + b1_[1]

    sc_re = 1.0 + fo_re[:, :EMBED].reshape(1, H, WF, BLOCKS, BS)
    sh_re = fo_re[:, EMBED:].reshape(1, H, WF, BLOCKS, BS)
    sc_im = 1.0 + fo_im[:, :EMBED].reshape(1, H, WF, BLOCKS, BS)
    sh_im = fo_im[:, EMBED:].reshape(1, H, WF, BLOCKS, BS)

    n_re = o1_re * sc_re - o1_im * sc_im + sh_re
    n_im = o1_im * sc_re + o1_re * sc_im + sh_im
    o1_re = np.maximum(n_re, 0.0)
    o1_im = np.maximum(n_im, 0.0)

    o2_re = _softshrink(_blockmm(o1_re, w2_[0]) - _blockmm(o1_im, w2_[1]) + b2_[0], LAMBD)
    o2_im = _softshrink(_blockmm(o1_im, w2_[0]) + _blockmm(o1_re, w2_[1]) + b2_[1], LAMBD)

    spec = (o2_re + 1j * o2_im).reshape(H, WF, EMBED)
    filt = np.fft.irfft2(spec, s=(H, W), axes=(0, 1), norm="ortho").astype(np.float32)
    h_mid = filt[None] + xn + residual  # filter bias (xn) + double_skip residual

    # ---- host: second half (device did scale/shift) ----
    h2 = _layernorm(h_mid, np.asarray(norm2_w, np.float32), np.asarray(norm2_b, np.float32))
    scale = 1.0 + ss_mlp[:, :LATENT].reshape(1, H, W, LATENT)
    shift = ss_mlp[:, LATENT:].reshape(1, H, W, LATENT)
    hh = h2.reshape(H * W, EMBED) @ np.asarray(fc1_w, np.float32).T + np.asarray(fc1_b, np.float32)
    hh = hh.reshape(1, H, W, LATENT) * scale + shift
    hh = _gelu(hh)
    out = hh.reshape(H * W, LATENT) @ np.asarray(fc2_w, np.float32).T + np.asarray(fc2_b, np.float32)
    return (out.reshape(1, H, W, EMBED) + h_mid).astype(np.float32)
